# revision 60
# baseline (speedup 1.0000x reference)
"""GCN encoder (2-layer GCNConv + LayerNorm + ELU + residual) on 8 Trainium2
NeuronCores via Bass/Tile.

Strategy: partition nodes across the 8 cores by id (6250 each, padded to
6272 = 49 tiles). Each core aggregates the edges whose dst lands in its
partition. The gather source is a DRAM table h' = dinv * (x @ W) over all
nodes (replicated dense compute for layer 1; AllGather of per-core chunks
for layer 2). Aggregation = dma_gather of fp16 256B rows + PSUM-accumulating
matmuls:
  - "lo" stream (srcs on cores 0-4): per-node slot-aligned layout, B = I
  - "hi" stream (srcs on cores 5-7, window-local idx): densely packed,
    per-chunk staircase selection matrix built on-device with is_equal
Self-loops never enter the gather streams: the self message dinv^2*h[v] is
added in the epilogue from a resident own-rows SBUF buffer.

Tables are PARTITION-MAJOR per window (row = 1 + m*WTILES + wtile for L1,
row = 1 + c*6272 + m*49 + t for L2) so dense-phase table writes emit 4KB+
descriptors instead of one 256B descriptor per row. Layer 1 and layer 2
use separate idx value arrays (same slot layout) because the two table
layouts differ.

All graph preprocessing happens on host in numpy; all float math on device.
"""
import sys

sys.path.insert(0, "/opt/trn_rl_repo")

import numpy as np

N_NODES = 50000
N_EDGES = 800000
IN_DIM = 256
F = 128
NCORES = 8
NP = N_NODES // NCORES          # 6250 nodes per core
TILES = 50                      # dst tiles per core (50 so the lo/hi tile
                                # split has room: 31 lo + 19 hi tiles)
NPAD = TILES * 128              # 6400
GT = NCORES * TILES             # 400 padded global tiles
DENSE_PAD = GT * 128            # 51200 padded positions
LOT = 250                       # L1 lo-window tiles (node id < 31250)
HIT = 150                       # L1 hi-window tiles
LO1_ROWS = 1 + 128 * LOT        # 32001 rows in L1 lo table (row 0 = Z)
HI1_ROWS = 128 * HIT            # 19200 rows in L1 hi table
# L2 table is chunk-major: AGC_T tile-range chunks, each rank-major inside,
# so each chunk is a contiguous AllGather output region
T_LO = 31                       # lo nodes (id<31250) live in tiles [0,31)
AGC_T = ((0, 13), (13, 26), (26, 31), (31, 41), (41, 50))
LO2_ROWS = 1 + NCORES * 128 * T_LO          # 31745 (incl leading Z)
T2ROWS = 1 + DENSE_PAD + 1                  # 51202
HI_BASE2 = LO2_ROWS
CALLCOLS = 8                    # slot cols per gather call (1024 idxs)

_CACHE = {}


def _cumcount(key):
    """0-based running count within equal-valued runs of a sorted key."""
    n = len(key)
    if n == 0:
        return np.zeros(0, dtype=np.int64)
    first = np.r_[True, key[1:] != key[:-1]]
    start = np.maximum.accumulate(np.where(first, np.arange(n), 0))
    return np.arange(n) - start


# ----------------------------------------------------------------- host prep
def _prep(x, W1, b1, g1, be1, W2, b2, g2, be2, edge_index):
    src = np.asarray(edge_index[0], dtype=np.int64)
    dst = np.asarray(edge_index[1], dtype=np.int64)
    x = np.asarray(x)

    deg = np.bincount(dst, minlength=N_NODES).astype(np.float64) + 1.0
    dinv = (1.0 / np.sqrt(deg)).astype(np.float32)

    # streams exclude self-loops (self message added from resident own rows)
    s_all, d_all = src, dst
    # L1 table-window membership is FIXED by node id (decoupled from core
    # assignment) so lo/hi degrees stay stable under rebalancing
    NLO = 31250
    is_lo_edge = s_all < NLO

    lodeg_n = np.bincount(d_all[is_lo_edge], minlength=N_NODES)
    hideg_n = np.bincount(d_all[~is_lo_edge], minlength=N_NODES)
    # assignment: global lo-degree sort -> 1024-node windows -> within each
    # window assign cores round-robin on descending hi-degree (balances
    # per-(core,tile) hi counts, so H_hi = max_c hicnt is tight)
    gorder = np.argsort(-lodeg_n, kind="stable")
    core = np.empty(N_NODES, dtype=np.int64)
    rank = np.empty(N_NODES, dtype=np.int64)
    # lo-set nodes (id < 31250) land in TILES [0, T_LO) across all 8 cores;
    # hi-set nodes in tiles [T_LO, TILES). This makes the L2 lo/hi table
    # regions contiguous under the chunk-major layout below.
    lo_nodes = gorder[gorder < NLO]
    hi_nodes = gorder[gorder >= NLO]
    for nodes, trange in ((lo_nodes, range(T_LO)),
                          (hi_nodes, range(T_LO, TILES))):
        for i, t in enumerate(trange):
            w = nodes[i * 1024:(i + 1) * 1024]
            o = np.argsort(-hideg_n[w], kind="stable")
            k = np.arange(len(w))
            core[w[o]] = k % NCORES
            rank[w[o]] = t * 128 + k // NCORES
    node_tile = rank // 128                      # tile within core
    node_m = rank % 128                          # partition slot

    # L1 table rows (partition-major per window, window index = node id)
    nid = np.arange(N_NODES)
    r1 = np.where(
        nid < NLO,
        1 + (nid % 128) * LOT + nid // 128,
        ((nid - NLO) % 128) * HIT + (nid - NLO) // 128)  # hi: local, no Z
    # L1 dense position of each node (window-major, tile-padded)
    pos1 = np.where(nid < NLO, nid, LOT * 128 + (nid - NLO))
    # L2 table rows: chunk-major (AGC_T tile ranges), rank-major per chunk,
    # partition-major per rank piece: row = 1 + base_k + (c*128 + m)*len_k
    # + (t - t0_k)
    agc_len = np.array([t1 - t0 for t0, t1 in AGC_T])
    agc_base = np.r_[0, np.cumsum(NCORES * 128 * agc_len)][:-1]
    tile2chunk = np.zeros(TILES, dtype=np.int64)
    for k, (t0, t1) in enumerate(AGC_T):
        tile2chunk[t0:t1] = k
    ck = tile2chunk[node_tile]
    r2 = (1 + agc_base[ck] + (core * 128 + node_m) * agc_len[ck]
          + node_tile - np.array([t0 for t0, _ in AGC_T])[ck])

    # --- per-core, per-tile structure --------------------------------------
    maxlo = np.zeros((NCORES, TILES), dtype=np.int64)
    hicnt = np.zeros((NCORES, TILES), dtype=np.int64)
    for c in range(NCORES):
        sel = core == c
        np.maximum.at(maxlo[c], node_tile[sel], lodeg_n[sel])
        np.add.at(hicnt[c], node_tile[sel], hideg_n[sel])
    M_lo = np.maximum(maxlo.max(axis=0), 1)               # [TILES]
    H_hi = (hicnt.max(axis=0) + 127) // 128               # [TILES] chunks

    # --- gather-call layout (shared across cores) ---------------------------
    LOCUM = np.r_[0, np.cumsum(M_lo)]
    HICUM = np.r_[0, np.cumsum(H_hi)]
    LOTOT, HITOT = int(LOCUM[-1]), int(HICUM[-1])
    NCALL_LO = (LOTOT + CALLCOLS - 1) // CALLCOLS
    NCALL_HI = (HITOT + CALLCOLS - 1) // CALLCOLS
    IDXCOLS = (NCALL_LO + NCALL_HI) * CALLCOLS * 8
    HICOLS = NCALL_HI * CALLCOLS

    struct = (
        tuple(int(v) for v in M_lo),
        tuple(int(v) for v in H_hi),
    )

    # --- per-core idx + dstrow arrays --------------------------------------
    idx_arrs, dstrow_arrs, dinv_own_arrs = [], [], []
    e_t = node_tile[d_all]
    e_m = node_m[d_all]
    e_core = core[d_all]
    ev_lo1 = r1[s_all]                  # L1 lo idx value
    ev_hi1 = r1[s_all]                  # L1 hi idx value (already local)
    ev_lo2 = r2[s_all]                  # L2 lo idx value
    ev_hi2 = r2[s_all] - HI_BASE2       # L2 hi idx value
    HIBASE = NCALL_LO * CALLCOLS * 128

    CC128 = CALLCOLS * 128

    def build_idx(lmask_idx, hmask_idx, lval, hval, lt, lm, p_cnt, ht, hm_s,
                  j_cnt):
        idx_big = np.zeros(IDXCOLS * 16, dtype=np.int16)
        g = LOCUM[lt] + p_cnt
        fl = CC128 * (g // CALLCOLS) + 128 * (g % CALLCOLS) + lm
        idx_big[fl] = lval.astype(np.int16)
        g = HICUM[ht] + j_cnt // 128
        m_slot = j_cnt % 128
        fl = HIBASE + CC128 * (g // CALLCOLS) + 128 * (g % CALLCOLS) + m_slot
        idx_big[fl] = hval.astype(np.int16)
        idx_2d = idx_big.reshape(IDXCOLS, 16).T.copy()
        return np.tile(idx_2d, (8, 1))

    for c in range(NCORES):
        emask = e_core == c
        # ---------- lo stream ordering
        lmask = emask & is_lo_edge
        lt, lm = e_t[lmask], e_m[lmask]
        lv1, lv2 = ev_lo1[lmask], ev_lo2[lmask]
        o = np.argsort(lt * 128 + lm, kind="stable")
        lt, lm, lv1, lv2 = lt[o], lm[o], lv1[o], lv2[o]
        p_cnt = _cumcount(lt * 128 + lm)
        # ---------- hi stream ordering
        hmask = emask & ~is_lo_edge
        ht, hm = e_t[hmask], e_m[hmask]
        hv1, hv2 = ev_hi1[hmask], ev_hi2[hmask]
        o = np.argsort(ht * 128 + hm, kind="stable")
        ht, hm, hv1, hv2 = ht[o], hm[o], hv1[o], hv2[o]
        j_cnt = _cumcount(ht)

        idx1 = build_idx(None, None, lv1, hv1, lt, lm, p_cnt, ht, hm, j_cnt)
        idx2 = build_idx(None, None, lv2, hv2, lt, lm, p_cnt, ht, hm, j_cnt)
        idx_arrs.append(np.concatenate([idx1, idx2], axis=1))

        dstrow = np.full((max(HICOLS, 1) * 128,), 128, dtype=np.int64)
        g = HICUM[ht] + j_cnt // 128
        m_slot = j_cnt % 128
        dstrow[g * 128 + m_slot] = hm
        # B selection matrices, one [128 msg x 128 slot] per hi col, packed
        # as [128 part(msg j), HICOLS*128] (col-major groups)
        eye = np.eye(129, 128, dtype=np.float16)
        bm = eye[dstrow.reshape(max(HICOLS, 1), 128)]      # [HICOLS,128,128]
        dstrow_arrs.append(
            np.ascontiguousarray(bm.transpose(1, 0, 2).reshape(
                128, max(HICOLS, 1) * 128)))

        dv = np.zeros((128, TILES), np.float32)
        own = np.where(core == c)[0]
        dv[node_m[own], node_tile[own]] = dinv[own]
        dinv_own_arrs.append(dv)

    # --- dense-phase inputs -------------------------------------------------
    # padded position -> node id (-1 for pad slots)
    pos_node = np.full(DENSE_PAD, -1, dtype=np.int64)
    pos_node[pos1] = np.arange(N_NODES)
    valid = pos_node >= 0
    xT = np.zeros((IN_DIM, DENSE_PAD), dtype=np.float16)
    xT[:, valid] = x[pos_node[valid]].T.astype(np.float16)
    dinv_d1 = np.zeros((128, GT), np.float32)
    pm = np.arange(DENSE_PAD)
    dinv_d1[pm % 128, pm // 128] = np.where(valid, dinv[pos_node % N_NODES],
                                            0.0)

    xT_own_arrs = []
    for c in range(NCORES):
        xo = np.zeros((IN_DIM, NPAD), dtype=np.float16)
        own = np.where(core == c)[0]
        xo[:, rank[own]] = x[own].T.astype(np.float16)
        xT_own_arrs.append(xo)

    rep = lambda v: np.ascontiguousarray(
        np.broadcast_to(np.asarray(v, np.float32), (128, F)))
    common = {
        "xT": xT,
        "W1": np.asarray(W1, np.float16),
        "W2h": np.asarray(W2, np.float16),
        "ident": np.eye(128, dtype=np.float16),
        "dinv_d1": dinv_d1,
        "b1r": rep(b1), "g1r": rep(g1), "be1r": rep(be1),
        "b2r": rep(b2), "g2r": rep(g2), "be2r": rep(be2),
    }
    in_maps = []
    for c in range(NCORES):
        m = dict(common)
        m["idx"] = idx_arrs[c]
        m["bmat"] = dstrow_arrs[c]
        m["dinv_own"] = dinv_own_arrs[c]
        m["xT_own"] = xT_own_arrs[c]
        in_maps.append(m)
    # map core-order output rows back to node ids: core c tile t slot m
    own_node = np.full((NCORES, NPAD), -1, dtype=np.int64)
    for c in range(NCORES):
        own = np.where(core == c)[0]
        own_node[c, node_tile[own] * 128 + node_m[own]] = own
    return struct, in_maps, own_node


# ------------------------------------------------------------- build program
def _build(struct, phase=4):
    import concourse.bass as bass
    import concourse.mybir as mybir
    from concourse import bacc, tile

    M_lo, H_hi = struct
    dt = mybir.dt
    AF = mybir.ActivationFunctionType
    OP = mybir.AluOpType
    LOCUM = np.r_[0, np.cumsum(M_lo)].astype(int)
    HICUM = np.r_[0, np.cumsum(H_hi)].astype(int)
    NCALL_LO = (int(LOCUM[-1]) + CALLCOLS - 1) // CALLCOLS
    NCALL_HI = (int(HICUM[-1]) + CALLCOLS - 1) // CALLCOLS
    IDXCOLS = (NCALL_LO + NCALL_HI) * CALLCOLS * 8
    HICOLS = NCALL_HI * CALLCOLS
    HIIDXBASE = NCALL_LO * CALLCOLS * 8
    HICOLS_IN = max(HICOLS, 1)

    nc = bacc.Bacc("TRN2", target_bir_lowering=False, debug=False,
                   num_devices=NCORES, num_swdge_queues=4,
                   dynamic_dma_scratch_size=32768)
    inp = lambda n, s, d: nc.dram_tensor(n, s, d, kind="ExternalInput")
    xT = inp("xT", [IN_DIM, DENSE_PAD], dt.float16)
    xT_own = inp("xT_own", [IN_DIM, NPAD], dt.float16)
    W1 = inp("W1", [IN_DIM, F], dt.float16)
    W2h = inp("W2h", [F, F], dt.float16)
    ident = inp("ident", [128, 128], dt.float16)
    dinv_d1 = inp("dinv_d1", [128, GT], dt.float32)
    dinv_own = inp("dinv_own", [128, TILES], dt.float32)
    idx_in = inp("idx", [128, 2 * IDXCOLS], dt.int16)
    bmat_in = inp("bmat", [128, HICOLS_IN * 128], dt.float16)
    b1r = inp("b1r", [128, F], dt.float32)
    g1r = inp("g1r", [128, F], dt.float32)
    be1r = inp("be1r", [128, F], dt.float32)
    b2r = inp("b2r", [128, F], dt.float32)
    g2r = inp("g2r", [128, F], dt.float32)
    be2r = inp("be2r", [128, F], dt.float32)
    out_own = nc.dram_tensor("out_own", [NPAD, F], dt.float32,
                             kind="ExternalOutput")

    h1_lo = nc.dram_tensor("h1_lo", [LO1_ROWS, F], dt.float16)
    h1_hi = nc.dram_tensor("h1_hi", [HI1_ROWS, F], dt.float16)
    # per-chunk own pieces (row = m*len_k + (t-t0)) so each AllGather chunk
    # fires as soon as its tile range is done (pipelines under L1 agg)
    h2_own_ks = [
        nc.dram_tensor(f"h2_own{k}", [128 * (t1 - t0), F], dt.float16)
        for k, (t0, t1) in enumerate(AGC_T)
    ]
    h2_tab = nc.dram_tensor("h2_tab", [T2ROWS, F], dt.float16,
                            addr_space="Shared")

    with tile.TileContext(nc) as tc:
        cst = tc.alloc_tile_pool(name="cst", bufs=1)
        res = tc.alloc_tile_pool(name="res", bufs=1)

        ident_t = cst.tile([128, 128], dt.float16)
        nc.sync.dma_start(ident_t[:], ident[:, :])
        W1_t = cst.tile([128, 2, F], dt.float16)
        for kc in range(2):
            nc.sync.dma_start(W1_t[:, kc, :], W1[kc * 128:(kc + 1) * 128, :])
        W2_t = cst.tile([128, F], dt.float16)
        nc.sync.dma_start(W2_t[:], W2h[:, :])
        dinvd1_t = cst.tile([128, GT], dt.float32)
        nc.sync.dma_start(dinvd1_t[:], dinv_d1[:, :])
        dinvo_t = cst.tile([128, TILES], dt.float32)
        nc.sync.dma_start(dinvo_t[:], dinv_own[:, :])
        idx_t = cst.tile([128, 2 * IDXCOLS], dt.int16)
        # L1 idx half loads now; the L2 half is deferred below dense1 so it
        # doesn't compete with the startup-critical x reads / table writes
        nc.sync.dma_start(idx_t[:, 0:IDXCOLS], idx_in[:, 0:IDXCOLS])
        bias_ts = {}
        for nm, ap_ in (("b1", b1r), ("g1", g1r), ("be1", be1r),
                        ("b2", b2r), ("g2", g2r), ("be2", be2r)):
            t = cst.tile([128, F], dt.float32, tag=f"cst_{nm}")
            nc.sync.dma_start(t[:], ap_[:, :])
            bias_ts[nm] = t
        eps_t = cst.tile([128, 1], dt.float32)
        nc.vector.memset(eps_t[:], 1e-5)
        neg1r = cst.tile([128, F], dt.float32)
        nc.vector.memset(neg1r[:], -1.0)
        zeror = cst.tile([128, F], dt.float32)
        nc.vector.memset(zeror[:], 0.0)
        zrow = cst.tile([128, F], dt.float16)
        nc.vector.memset(zrow[:], 0.0)
        # Z rows
        nc.sync.dma_start(h1_lo[0:1, :], zrow[:1, :])
        nc.sync.dma_start(h2_tab[0:1, :], zrow[:1, :])
        nc.sync.dma_start(h2_tab[T2ROWS - 1:T2ROWS, :], zrow[:1, :])

        # resident layer-1 activations (f16; also the residual source)
        y1h_all = res.tile([128, TILES, F], dt.float16)
        # resident own-node table rows (self messages, no gather needed)
        h1own_all = res.tile([128, TILES, F], dt.float16)
        h2own_all = res.tile([128, TILES, F], dt.float16)

        # agg pools hoisted above dense1 pools so the agg gathers carry no
        # WAR dependency on dense1's released SBUF
        msgp = tc.alloc_tile_pool(name="msg", bufs=12)
        bqp = tc.alloc_tile_pool(name="bq", bufs=3)
        agp = tc.alloc_tile_pool(name="agp", bufs=3, space="PSUM")
        epp = tc.alloc_tile_pool(name="ep", bufs=3)
        d2 = d2p = None
        if phase >= 3:
            d2 = tc.alloc_tile_pool(name="d2", bufs=3)
            d2p = tc.alloc_tile_pool(name="d2p", bufs=1, space="PSUM")

        # ------------------------------------------------ dense 1: h1' table
        XG = 16
        lo_pm = h1_lo[1:, :].rearrange("(m g) f -> m g f", m=128)
        hi_pm = h1_hi[:, :].rearrange("(m g) f -> m g f", m=128)

        def write_slab(slab, g0, ng):
            # slab [128, ng, F]: global tiles [g0, g0+ng) partition-major
            g1_ = g0 + ng
            if g1_ <= LOT:
                nc.sync.dma_start(lo_pm[:, g0:g1_, :], slab[:, :ng, :])
            elif g0 >= LOT:
                nc.sync.dma_start(hi_pm[:, g0 - LOT:g1_ - LOT, :],
                                  slab[:, :ng, :])
            else:
                k = LOT - g0
                nc.sync.dma_start(lo_pm[:, g0:LOT, :], slab[:, :k, :])
                nc.sync.dma_start(hi_pm[:, 0:g1_ - LOT, :], slab[:, k:ng, :])

        with (
            tc.tile_pool(name="d1", bufs=3) as d1,
            tc.tile_pool(name="d1p", bufs=3, space="PSUM") as d1p,
        ):
            # own-node pass FIRST: h1own = dinv_own * (x_own @ W1), stays in
            # SBUF. Running it first means the agg self-add matmuls for the
            # first tiles never stall on it.
            for g in range(0, TILES, XG):
                gtiles = range(g, min(g + XG, TILES))
                ncols = 128 * len(gtiles)
                xs = d1.tile([128, 2, ncols], dt.float16, tag="xs")
                for kc in range(2):
                    nc.sync.dma_start(
                        xs[:, kc, :],
                        xT_own[kc * 128:(kc + 1) * 128,
                               g * 128:g * 128 + ncols])
                for j, t in enumerate(gtiles):
                    ps = d1p.tile([128, F], dt.float32, tag="psd1")
                    for kc in range(2):
                        nc.tensor.matmul(
                            out=ps[:], lhsT=xs[:, kc, bass.ts(j, 128)],
                            rhs=W1_t[:, kc, :],
                            start=(kc == 0), stop=(kc == 1))
                    nc.scalar.activation(h1own_all[:, t, :], ps[:], AF.Copy,
                                         scale=dinvo_t[:, t:t + 1])

            for g in range(0, GT, XG):
                gtiles = range(g, min(g + XG, GT))
                ncols = 128 * len(gtiles)
                xs = d1.tile([128, 2, ncols], dt.float16, tag="xs")
                for kc in range(2):
                    nc.sync.dma_start(
                        xs[:, kc, :],
                        xT[kc * 128:(kc + 1) * 128, g * 128:g * 128 + ncols])
                slab = d1.tile([128, len(gtiles), F], dt.float16, tag="hslab")
                for j, t in enumerate(gtiles):
                    ps = d1p.tile([128, F], dt.float32, tag="psd1")
                    for kc in range(2):
                        nc.tensor.matmul(
                            out=ps[:], lhsT=xs[:, kc, bass.ts(j, 128)],
                            rhs=W1_t[:, kc, :],
                            start=(kc == 0), stop=(kc == 1))
                    # split the dinv-scale+cast between ACT and DVE so
                    # neither engine serializes the dense phase
                    if t % 2 == 0:
                        nc.scalar.activation(slab[:, j, :], ps[:], AF.Copy,
                                             scale=dinvd1_t[:, t:t + 1])
                    else:
                        nc.vector.tensor_scalar_mul(slab[:, j, :], ps[:],
                                                    dinvd1_t[:, t:t + 1])
                write_slab(slab, g, len(gtiles))

        # ---------------------------------------------------- aggregation fn
        qctr = [0]

        def agg_layer(lo_ap, hi_ap, own, idxbase, bname, gname, bename, post):
            ep = epp
            bufs = {}

            LOTOT_, HITOT_ = int(LOCUM[-1]), int(HICUM[-1])

            def rhs_col(stream, g):
                ci = g // CALLCOLS
                key = (stream, ci)
                if key not in bufs:
                    mt = msgp.tile([128, CALLCOLS, F], dt.float16,
                                   tag=f"m{stream}")
                    base = idxbase + (0 if stream == "lo" else HIIDXBASE) \
                        + ci * CALLCOLS * 8
                    # trailing call of each stream issues only its real
                    # columns (desc-gen cost is per idx, padded or not)
                    tot = LOTOT_ if stream == "lo" else HITOT_
                    ncols = min(CALLCOLS, tot - ci * CALLCOLS)
                    nc.gpsimd.dma_gather(
                        out_ap=mt[:, 0:ncols, :],
                        in_ap=lo_ap if stream == "lo" else hi_ap,
                        idxs_ap=idx_t[:, base:base + ncols * 8],
                        num_idxs=ncols * 128,
                        num_idxs_reg=ncols * 128,
                        elem_size=F,
                        single_packet=False,
                        queue_num=qctr[0] % 4,
                    )
                    qctr[0] += 1
                    bufs[key] = mt
                return bufs[key][:, g % CALLCOLS, :]

            for t in range(TILES):
                ps = agp.tile([128, F], dt.float32, tag="psag")
                nlo, nhi = M_lo[t], H_hi[t]
                for p in range(nlo):
                    nc.tensor.matmul(
                        out=ps[:], lhsT=ident_t[:],
                        rhs=rhs_col("lo", LOCUM[t] + p),
                        start=(p == 0), stop=False)
                for q in range(nhi):
                    gcol = HICUM[t] + q
                    gi = gcol // CALLCOLS
                    if ("bq", gi) not in bufs:
                        bt = bqp.tile([128, CALLCOLS, 128], dt.float16,
                                      tag="bqt")
                        ncols = min(CALLCOLS, HICOLS - gi * CALLCOLS)
                        nc.sync.dma_start(
                            bt[:, 0:ncols, :],
                            bmat_in[:, gi * CALLCOLS * 128:
                                    (gi * CALLCOLS + ncols) * 128])
                        bufs[("bq", gi)] = bt
                    bq = bufs[("bq", gi)][:, gcol % CALLCOLS, :]
                    nc.tensor.matmul(
                        out=ps[:], lhsT=bq,
                        rhs=rhs_col("hi", gcol),
                        start=False, stop=False)
                # self message: PSUM += I @ (dinv*h_own)
                nc.tensor.matmul(out=ps[:], lhsT=ident_t[:],
                                 rhs=own[:, t, :], start=False, stop=True)
                # -------- epilogue: scale, bias, LN (bn_stats), ELU
                z = ep.tile([128, F], dt.float32, tag="z")
                nc.scalar.activation(z[:], ps[:], AF.Copy,
                                     scale=dinvo_t[:, t:t + 1])
                nc.vector.tensor_tensor(out=z[:], in0=z[:],
                                        in1=bias_ts[bname][:], op=OP.add)
                st6 = ep.tile([128, 6], dt.float32, tag="st6")
                nc.vector.bn_stats(st6[:], z[:])
                st2 = ep.tile([128, 2], dt.float32, tag="st2")
                nc.vector.bn_aggr(st2[:], st6[:])
                sd = ep.tile([128, 1], dt.float32, tag="sd")
                nc.scalar.activation(sd[:], st2[:, 1:2], AF.Sqrt,
                                     bias=eps_t[:, 0:1])
                inv = ep.tile([128, 1], dt.float32, tag="inv")
                nc.vector.reciprocal(inv[:], sd[:])
                nmi = ep.tile([128, 1], dt.float32, tag="nmi")
                nc.vector.tensor_scalar(nmi[:], st2[:, 0:1], inv[:, 0:1],
                                        -1.0, OP.mult, OP.mult)
                # zn = (z - mean) * inv = z*inv + (-mean*inv)  (one ACT pass)
                zn = ep.tile([128, F], dt.float32, tag="zn")
                nc.scalar.activation(zn[:], z[:], AF.Identity,
                                     scale=inv[:, 0:1], bias=nmi[:, 0:1])
                nc.vector.tensor_tensor(out=zn[:], in0=zn[:],
                                        in1=bias_ts[gname][:], op=OP.mult)
                nc.vector.tensor_tensor(out=zn[:], in0=zn[:],
                                        in1=bias_ts[bename][:], op=OP.add)
                # ELU(x) = max(x, min(exp(x) - 1, 0))
                ex = ep.tile([128, F], dt.float32, tag="ex")
                nc.scalar.activation(ex[:], zn[:], AF.Exp)
                em = ep.tile([128, F], dt.float32, tag="em")
                nc.vector.tensor_tensor(out=em[:], in0=ex[:], in1=neg1r[:],
                                        op=OP.add)
                nc.vector.tensor_tensor(out=em[:], in0=em[:], in1=zeror[:],
                                        op=OP.min)
                post(t, zn, em)

        # dense-2: each tile's h2' row block is computed as soon as its y1
        # lands; h2_own written in partition-major batches
        def dense2_tile(t):
            trp = d2p.tile([128, 128], dt.float16, tag="trp")
            nc.tensor.transpose(out=trp[:], in_=y1h_all[:, t, :],
                                identity=ident_t[:])
            y1T = d2.tile([128, 128], dt.float16, tag="y1T")
            nc.vector.tensor_copy(y1T[:], trp[:])
            ps2 = d2p.tile([128, F], dt.float32, tag="ps2")
            nc.tensor.matmul(out=ps2[:], lhsT=y1T[:], rhs=W2_t[:],
                             start=True, stop=True)
            nc.scalar.activation(h2own_all[:, t, :], ps2[:], AF.Copy,
                                 scale=dinvo_t[:, t:t + 1])
            for k, (t0, t1_) in enumerate(AGC_T):
                if t == t1_ - 1:
                    pm = h2_own_ks[k][:, :].rearrange(
                        "(m tk) f -> m tk f", m=128)
                    nc.sync.dma_start(pm[:, :, :], h2own_all[:, t0:t1_, :])

        # layer-1 post: y1 = max(zn, min(exp(zn)-1, 0)) = ELU, stored f16
        def post1(t, zn, em):
            nc.vector.tensor_tensor(out=y1h_all[:, t, :], in0=zn[:],
                                    in1=em[:], op=mybir.AluOpType.max)
            if phase >= 3:
                dense2_tile(t)

        nc.sync.dma_start(idx_t[:, IDXCOLS:2 * IDXCOLS],
                          idx_in[:, IDXCOLS:2 * IDXCOLS])

        if phase >= 2:
            agg_layer(h1_lo[:, :], h1_hi[:, :], h1own_all, 0,
                      "b1", "g1", "be1", post1)

        # ------------------------------------------- AllGather (per chunk)
        if phase >= 3:
            base = 1
            for k, (t0, t1_) in enumerate(AGC_T):
                ln = NCORES * 128 * (t1_ - t0)
                nc.gpsimd.collective_compute(
                    "AllGather", mybir.AluOpType.bypass,
                    replica_groups=[list(range(NCORES))],
                    ins=[h2_own_ks[k].ap().opt()],
                    outs=[h2_tab[base:base + ln, :].opt()],
                )
                base += ln

        # ------------------------------------------------ layer 2 + residual
        if phase >= 4:
            with tc.tile_pool(name="fin", bufs=3) as fin:
                def post2(t, zn, em):
                    y2 = fin.tile([128, F], dt.float32, tag="y2")
                    nc.vector.tensor_tensor(out=y2[:], in0=zn[:], in1=em[:],
                                            op=mybir.AluOpType.max)
                    nc.vector.tensor_tensor(out=y2[:], in0=y2[:],
                                            in1=y1h_all[:, t, :],
                                            op=mybir.AluOpType.add)
                    nc.sync.dma_start(out_own[t * 128:(t + 1) * 128, :],
                                      y2[:])

                agg_layer(h2_tab[0:LO2_ROWS, :], h2_tab[HI_BASE2:T2ROWS, :],
                          h2own_all, IDXCOLS, "b2", "g2", "be2", post2)

        if d2p is not None:
            d2p.release()
            d2.release()
        epp.release()
        agp.release()
        bqp.release()
        msgp.release()
        res.release()
        cst.release()

    nc.compile()
    return nc


# ------------------------------------------------------------------- driver
def _run(inputs, trace=False, phase=4):
    from concourse.bass_utils import run_bass_kernel_spmd

    struct, in_maps, own_node = _prep(**inputs)
    key = (hash(struct), phase)
    if key not in _CACHE:
        _CACHE[key] = _build(struct, phase=phase)
    nc = _CACHE[key]
    res = run_bass_kernel_spmd(nc, in_maps, core_ids=list(range(NCORES)),
                               trace=trace)
    out = np.empty((N_NODES, F), dtype=np.float32)
    for c in range(NCORES):
        rows = res.results[c]["out_own"]
        sel = own_node[c] >= 0
        out[own_node[c, sel]] = rows[sel]
    return out, res


def kernel(**inputs):
    out, _ = _run(inputs, trace=False)
    return out

